# revision 15
# baseline (speedup 1.0000x reference)
"""Levina-Bickel MLE intrinsic-dimension kernel for Trainium2 (8 NeuronCores).

Problem: X [B=4, N=8192, D=32] f32, k=16.
  d2[b,i,j] = |x_i - x_j|^2 ; per row take 16 smallest (incl. self), drop self,
  s_i = sum_j log(d_16/d_j), out[b] = 14*N / sum_i s_i.

Sharding: core c -> batch c//2, query rows (c%2)*4096 ..+4096, full X[b]
replicated as the matmul moving operand.

v2 design (vs v1 = 8x max8-PSUM per tile, DVE-bound at ~10.4us/tile):
  The matmul emits y = SHIFT - d2 directly (bias rows folded into the K=100
  bf16 hi/lo contraction).  Per 128-row tile, 4 units of 2048 keys each:
  ACT casts the [128,2048] f32 PSUM tile to fp16 SBUF (1.97us, off the DVE),
  DVE folds pairwise max twice at 2x_1P (stride-1024 then stride-512, 4:1
  fold, 0.7+0.43us) and runs max8 on the two 256-wide halves (cov-1024
  blocks).  64 candidates/tile -> top-16 via max8 + is_lt-mask + max8.
  Selected y values land in a per-core [128, 32*16] buffer; ONE Ln pass +
  3D-AP tensor_reduce at the end compute s' = 15*L16 - sum(L) for all tiles
  (removes the per-tile ACT ops of v1).  Engine budget/tile: ACT 7.9us,
  DVE ~8.7us, PE ~7.3us (bf16 matmuls at 1.2GHz pipeline back-to-back).
  fp16 (not bf16) casts keep y quantization at 2^-5 => rel err ~2.1e-3
  (matches numpy sim), dominated by 4:1 fold collisions (4.4% of rows lose
  one neighbor to a same-slot collision).
  Measured: 277.6us HW exec on 8 cores (v1 was 325.8us); body is Scalar
  (cast)-bound at ~1.96us/unit with DVE at ~96% and the PE overlapped.
  Dead ends (measured): TT(PSUM,PSUM) is rejected by the backend (one PSUM
  read port); max8 has no 2x/4x uops (always 1 elem/lane/cycle); the PE
  never exceeds 1.2GHz here (622ns/512-col matmul incl fill+drain, 454
  back-to-back); GPSIMD tensor ops don't compile; casting via X-units
  (TT(PSUM,SBUF)) stalls the PE ~1.5us per use through PSUM hold time.
"""

import sys

sys.path.insert(0, "/opt/trn_rl_repo")

import numpy as np
import ml_dtypes

import concourse.bass as bass  # noqa: F401  (registers bass types)
import concourse.bacc as bacc
import concourse.tile as tile
import concourse.mybir as mybir
from concourse.bass_utils import run_bass_kernel_spmd

BF16 = ml_dtypes.bfloat16
B, N, D, KNN = 4, 8192, 32, 16
NCORES = 8
ROWS_PER_CORE = B * N // NCORES      # 4096
TILES = ROWS_PER_CORE // 128         # 32
NUNIT = 4                            # 2048-wide units per tile
SHIFT = 64.0                         # y = SHIFT - d2; top-16 y stay > 0
CLAMP = SHIFT - 0.25                 # Ln input floor: d2_self -> 0.25
LN_SELF = float(np.log(0.25))        # ln of clamped self distance

_compiled = None


def _build():
    nc = bacc.Bacc("TRN2", target_bir_lowering=False, debug=False)
    f32 = mybir.dt.float32
    bf16 = mybir.dt.bfloat16
    fp16 = mybir.dt.float16

    xt_d = nc.dram_tensor("xt", [128, N], bf16, kind="ExternalInput")
    qt_d = nc.dram_tensor("qt", [128, ROWS_PER_CORE], bf16, kind="ExternalInput")
    out_d = nc.dram_tensor("acc_out", [128, TILES], f32, kind="ExternalOutput")

    with tile.TileContext(nc) as tc:
        with (
            tc.tile_pool(name="persist", bufs=1) as persist,
            tc.tile_pool(name="psum", bufs=2, space="PSUM") as psum_pool,
            tc.tile_pool(name="work", bufs=3) as work,
        ):
            xt = persist.tile([128, N], bf16)
            qt = persist.tile([128, ROWS_PER_CORE], bf16)
            selbuf = persist.tile([128, TILES * 16], f32)
            selc = persist.tile([128, TILES * 16], f32)
            logs = persist.tile([128, TILES * 16], f32)
            shiftc = persist.tile([128, 1], f32)
            nc.vector.memset(shiftc[:], SHIFT)

            # tile 0's weights + first unit's keys land first, split across
            # DMA queues, so the pipeline can start while the rest streams in
            engs = (nc.scalar, nc.sync)
            nc.scalar.dma_start(xt[:, 0:512], xt_d.ap()[:, 0:512])
            nc.sync.dma_start(qt[:, 0:128], qt_d.ap()[:, 0:128])
            for h in range(1, 16):
                engs[h % 2].dma_start(xt[:, h * 512 : (h + 1) * 512],
                                      xt_d.ap()[:, h * 512 : (h + 1) * 512])
                if h == 5:
                    nc.sync.dma_start(qt[:, 128:], qt_d.ap()[:, 128:])

            # hoist the Ln table load into the DMA wait (it costs ~2.7us on
            # the Scalar queue the first time a table set is used)
            warm = persist.tile([128, 1], f32)
            warm2 = persist.tile([128, 1], f32)
            nc.vector.memset(warm[:], 1.0)
            nc.scalar.activation(
                warm2[:], warm[:], mybir.ActivationFunctionType.Ln
            )

            def merge(t, cands):
                """Top-16 of the 64 unit candidates -> selbuf[:, t*16:+16]."""
                s0 = t * 16
                nc.vector.max(selbuf[:, s0 : s0 + 8], cands[:])
                cands2 = work.tile([128, NUNIT * 16], f32, tag="c2", name="c2")
                # candidates are > 0 (top-16 y all positive), so masking the
                # first 8 ranks to 0 drops them from the second max8
                nc.vector.scalar_tensor_tensor(
                    cands2[:], cands[:], selbuf[:, s0 + 7 : s0 + 8], cands[:],
                    op0=mybir.AluOpType.is_lt, op1=mybir.AluOpType.mult,
                )
                nc.vector.max(selbuf[:, s0 + 8 : s0 + 16], cands2[:])

            pending = None
            for t in range(TILES):
                w = qt[:, t * 128 : (t + 1) * 128]
                cands = work.tile([128, NUNIT * 16], f32, tag="cands",
                                  name="cands")
                for u in range(NUNIT):
                    ps = psum_pool.tile([128, 2048], f32, tag="ps", name="ps")
                    for h in range(4):
                        c0 = u * 2048 + h * 512
                        nc.tensor.matmul(
                            ps[:, h * 512 : (h + 1) * 512],
                            w[0:100, :], xt[0:100, c0 : c0 + 512],
                            start=True, stop=True,
                        )
                    C = work.tile([128, 1024], fp16, tag="C", name="C")
                    S = work.tile([128, 2048], fp16, tag="S", name="S")
                    # Relu, not Identity: relu is filler in every ACT table
                    # set, so this avoids the Identity<->Ln table ping-pong
                    # (1.3us reload gating the first cast + another at the
                    # tail Ln).  Clamping far points' negative y to 0 is
                    # harmless: top-16 candidates are all > 0.
                    nc.scalar.activation(
                        S[:], ps[:], mybir.ActivationFunctionType.Relu
                    )
                    nc.vector.tensor_max(C[:], S[:, 0:1024], S[:, 1024:2048])
                    Dt = work.tile([128, 512], fp16, tag="D", name="D")
                    nc.vector.tensor_max(Dt[:], C[:, 0:512], C[:, 512:1024])
                    cb = u * 16
                    nc.vector.max(cands[:, cb : cb + 8], Dt[:, 0:256])
                    nc.vector.max(cands[:, cb + 8 : cb + 16], Dt[:, 256:512])
                    if u == 0 and pending is not None:
                        merge(*pending)
                        if t == TILES - 1:
                            # tiles 0..30 are final in selbuf: clamp them now
                            # so their Ln can run during the last tile's DVE
                            # chain instead of serializing after it
                            nc.vector.tensor_scalar_min(
                                selc[:, 0:496], selbuf[:, 0:496], CLAMP
                            )
                        pending = None
                pending = (t, cands)
            nc.scalar.activation(
                logs[:, 0:496], selc[:, 0:496],
                mybir.ActivationFunctionType.Ln, bias=shiftc[:], scale=-1.0,
            )
            merge(*pending)
            nc.vector.tensor_scalar_min(selc[:, 496:512], selbuf[:, 496:512],
                                        CLAMP)
            nc.scalar.activation(
                logs[:, 496:512], selc[:, 496:512],
                mybir.ActivationFunctionType.Ln, bias=shiftc[:], scale=-1.0,
            )
            logs3 = logs[:].rearrange("p (g j) -> p g j", j=16)
            rsum = persist.tile([128, TILES], f32)
            nc.vector.tensor_reduce(
                rsum[:], logs3, axis=mybir.AxisListType.X,
                op=mybir.AluOpType.add,
            )
            t15 = persist.tile([128, TILES], f32)
            nc.vector.tensor_scalar_mul(t15[:], logs3[:, :, 15], float(KNN - 1))
            acc = persist.tile([128, TILES], f32)
            nc.vector.tensor_sub(acc[:], t15[:], rsum[:])
            nc.sync.dma_start(out_d.ap()[:], acc[:])

    nc.compile()
    return nc


def get_compiled():
    global _compiled
    if _compiled is None:
        _compiled = _build()
    return _compiled


def _split(a):
    hi = a.astype(BF16)
    lo = (a - hi.astype(np.float64)).astype(BF16)
    return hi, lo


def prep_inputs(X):
    """X [B, N, D] f32 -> per-core {xt, qt} maps (y = SHIFT - d2 layout)."""
    in_maps = []
    for c in range(NCORES):
        b, h = c // 2, c % 2
        Xb = np.ascontiguousarray(X[b])                       # [N, D] f32
        sqx = (Xb.astype(np.float64) ** 2).sum(1)             # [N] f64
        Xhi, Xlo = _split(Xb)
        nhi, nlo = _split(-sqx)

        xt = np.zeros([128, N], BF16)
        xt[0:32] = (Xhi.astype(np.float32) * 2.0).astype(BF16).T
        xt[32:64] = (Xlo.astype(np.float32) * 2.0).astype(BF16).T
        xt[64:96] = xt[0:32]
        xt[96] = nhi
        xt[97] = nlo
        xt[98] = BF16(1.0)
        xt[99] = BF16(1.0)

        Qb = Xb[h * ROWS_PER_CORE : (h + 1) * ROWS_PER_CORE]  # [4096, D]
        sqq = sqx[h * ROWS_PER_CORE : (h + 1) * ROWS_PER_CORE]
        Qhi, Qlo = _split(Qb)
        bhi, blo = _split(SHIFT - sqq)
        qt = np.zeros([128, ROWS_PER_CORE], BF16)
        qt[0:32] = Qhi.T
        qt[32:64] = Qhi.T
        qt[64:96] = Qlo.T
        qt[96] = BF16(1.0)
        qt[97] = BF16(1.0)
        qt[98] = bhi
        qt[99] = blo

        in_maps.append({"xt": xt, "qt": qt})
    return in_maps


def finish(acc_list):
    """acc_list: per-core [128, TILES] f32 of 15*L16 - sum_{j=0..15} L_j.

    2*s_i = acc_i + ln(0.25)  (rank-0 is the clamped self distance).
    out_b = 2*(k-2)*N / sum_i 2*s_i ... i.e. (k-2)*N / sum s_i.
    """
    S = np.zeros(B, np.float64)
    for c, a in enumerate(acc_list):
        S[c // 2] += (a.astype(np.float64) + LN_SELF).sum()
    return (2.0 * (KNN - 2) * N / S).astype(np.float32)


def kernel(X, k):
    assert int(k) == KNN
    X = np.asarray(X, dtype=np.float32)
    assert X.shape == (B, N, D)
    nc = get_compiled()
    in_maps = prep_inputs(X)
    # The axon tunnel occasionally throws a transient
    # NRT_EXEC_UNIT_UNRECOVERABLE on execute; a retry reliably recovers.
    last_err = None
    for _ in range(3):
        try:
            res = run_bass_kernel_spmd(nc, in_maps, list(range(NCORES)))
            acc_list = [res.results[c]["acc_out"] for c in range(NCORES)]
            return finish(acc_list)
        except Exception as e:  # noqa: BLE001 - device transients surface broadly
            last_err = e
    raise last_err


# revision 17
# speedup vs baseline: 1.0073x; 1.0073x over previous
"""Levina-Bickel MLE intrinsic-dimension kernel for Trainium2 (8 NeuronCores).

Problem: X [B=4, N=8192, D=32] f32, k=16.
  d2[b,i,j] = |x_i - x_j|^2 ; per row take 16 smallest (incl. self), drop self,
  s_i = sum_j log(d_16/d_j), out[b] = 14*N / sum_i s_i.

Sharding: core c -> batch c//2, query rows (c%2)*4096 ..+4096, full X[b]
replicated as the matmul moving operand.

v2 design (vs v1 = 8x max8-PSUM per tile, DVE-bound at ~10.4us/tile):
  The matmul emits y = SHIFT - d2 directly (bias rows folded into the K=100
  bf16 hi/lo contraction).  Per 128-row tile, 4 units of 2048 keys each:
  ACT casts the [128,2048] f32 PSUM tile to fp16 SBUF (1.97us, off the DVE),
  DVE folds pairwise max twice at 2x_1P (stride-1024 then stride-512, 4:1
  fold, 0.7+0.43us) and runs max8 on the two 256-wide halves (cov-1024
  blocks).  64 candidates/tile -> top-16 via max8 + is_lt-mask + max8.
  Selected y values land in a per-core [128, 32*16] buffer; ONE Ln pass +
  3D-AP tensor_reduce at the end compute s' = 15*L16 - sum(L) for all tiles
  (removes the per-tile ACT ops of v1).  Engine budget/tile: ACT 7.9us,
  DVE ~8.7us, PE ~7.3us (bf16 matmuls at 1.2GHz pipeline back-to-back).
  fp16 (not bf16) casts keep y quantization at 2^-5 => rel err ~2.1e-3
  (matches numpy sim), dominated by 4:1 fold collisions (4.4% of rows lose
  one neighbor to a same-slot collision).
  Measured: 277.6us HW exec on 8 cores (v1 was 325.8us); body is Scalar
  (cast)-bound at ~1.96us/unit with DVE at ~96% and the PE overlapped.
  Dead ends (measured): TT(PSUM,PSUM) is rejected by the backend (one PSUM
  read port); max8 has no 2x/4x uops (always 1 elem/lane/cycle); the PE
  never exceeds 1.2GHz here (622ns/512-col matmul incl fill+drain, 454
  back-to-back); GPSIMD tensor ops don't compile; casting via X-units
  (TT(PSUM,SBUF)) stalls the PE ~1.5us per use through PSUM hold time.
"""

import sys

sys.path.insert(0, "/opt/trn_rl_repo")

import numpy as np
import ml_dtypes

import concourse.bass as bass  # noqa: F401  (registers bass types)
import concourse.bacc as bacc
import concourse.tile as tile
import concourse.mybir as mybir
from concourse.bass_utils import run_bass_kernel_spmd

BF16 = ml_dtypes.bfloat16
B, N, D, KNN = 4, 8192, 32, 16
NCORES = 8
ROWS_PER_CORE = B * N // NCORES      # 4096
TILES = ROWS_PER_CORE // 128         # 32
NUNIT = 4                            # 2048-wide units per tile
SHIFT = 64.0                         # y = SHIFT - d2; top-16 y stay > 0
CLAMP = SHIFT - 0.25                 # Ln input floor: d2_self -> 0.25
LN_SELF = float(np.log(0.25))        # ln of clamped self distance

_compiled = None


def _build():
    nc = bacc.Bacc("TRN2", target_bir_lowering=False, debug=False)
    f32 = mybir.dt.float32
    bf16 = mybir.dt.bfloat16
    fp16 = mybir.dt.float16

    xt_d = nc.dram_tensor("xt", [128, N], bf16, kind="ExternalInput")
    qt_d = nc.dram_tensor("qt", [128, ROWS_PER_CORE], bf16, kind="ExternalInput")
    out_d = nc.dram_tensor("acc_out", [128, TILES], f32, kind="ExternalOutput")

    with tile.TileContext(nc) as tc:
        with (
            tc.tile_pool(name="persist", bufs=1) as persist,
            tc.tile_pool(name="psum", bufs=2, space="PSUM") as psum_pool,
            tc.tile_pool(name="work", bufs=3) as work,
        ):
            xt = persist.tile([128, N], bf16)
            qt = persist.tile([128, ROWS_PER_CORE], bf16)
            selbuf = persist.tile([128, TILES * 16], f32)
            selc = persist.tile([128, TILES * 16], f32)
            logs = persist.tile([128, TILES * 16], f32)
            shiftc = persist.tile([128, 1], f32)
            nc.vector.memset(shiftc[:], SHIFT)

            # tile 0's weights + first unit's keys land first, split across
            # DMA queues, so the pipeline can start while the rest streams in
            nc.sync.dma_start(qt[:, 0:128], qt_d.ap()[:, 0:128])
            for h, eng in enumerate((nc.scalar, nc.gpsimd, nc.sync, nc.scalar)):
                eng.dma_start(xt[:, h * 512 : (h + 1) * 512],
                              xt_d.ap()[:, h * 512 : (h + 1) * 512])
            # unit 1's keys next (their late arrival stalled the pipeline
            # ~5us at tile 0), then the rest of qt, then units 2-3
            nc.scalar.dma_start(xt[:, 2048:3072], xt_d.ap()[:, 2048:3072])
            nc.sync.dma_start(xt[:, 3072:4096], xt_d.ap()[:, 3072:4096])
            nc.sync.dma_start(qt[:, 128:], qt_d.ap()[:, 128:])
            engs2 = (nc.scalar, nc.sync)
            for j in range(4):
                c0 = 4096 + j * 1024
                engs2[j % 2].dma_start(xt[:, c0 : c0 + 1024],
                                       xt_d.ap()[:, c0 : c0 + 1024])

            # hoist the Ln table load into the DMA wait (it costs ~2.7us on
            # the Scalar queue the first time a table set is used)
            warm = persist.tile([128, 1], f32)
            warm2 = persist.tile([128, 1], f32)
            nc.vector.memset(warm[:], 1.0)
            nc.scalar.activation(
                warm2[:], warm[:], mybir.ActivationFunctionType.Ln
            )

            def merge(t, cands):
                """Top-16 of the 64 unit candidates -> selbuf[:, t*16:+16]."""
                s0 = t * 16
                nc.vector.max(selbuf[:, s0 : s0 + 8], cands[:])
                cands2 = work.tile([128, NUNIT * 16], f32, tag="c2", name="c2")
                # candidates are > 0 (top-16 y all positive), so masking the
                # first 8 ranks to 0 drops them from the second max8
                nc.vector.scalar_tensor_tensor(
                    cands2[:], cands[:], selbuf[:, s0 + 7 : s0 + 8], cands[:],
                    op0=mybir.AluOpType.is_lt, op1=mybir.AluOpType.mult,
                )
                nc.vector.max(selbuf[:, s0 + 8 : s0 + 16], cands2[:])

            pending = None
            for t in range(TILES):
                w = qt[:, t * 128 : (t + 1) * 128]
                cands = work.tile([128, NUNIT * 16], f32, tag="cands",
                                  name="cands")
                for u in range(NUNIT):
                    ps = psum_pool.tile([128, 2048], f32, tag="ps", name="ps")
                    for h in range(4):
                        c0 = u * 2048 + h * 512
                        nc.tensor.matmul(
                            ps[:, h * 512 : (h + 1) * 512],
                            w[0:100, :], xt[0:100, c0 : c0 + 512],
                            start=True, stop=True,
                        )
                    C = work.tile([128, 1024], fp16, tag="C", name="C")
                    S = work.tile([128, 2048], fp16, tag="S", name="S")
                    nc.scalar.activation(
                        S[:], ps[:], mybir.ActivationFunctionType.Identity
                    )
                    nc.vector.tensor_max(C[:], S[:, 0:1024], S[:, 1024:2048])
                    Dt = work.tile([128, 512], fp16, tag="D", name="D")
                    nc.vector.tensor_max(Dt[:], C[:, 0:512], C[:, 512:1024])
                    cb = u * 16
                    nc.vector.max(cands[:, cb : cb + 8], Dt[:, 0:256])
                    nc.vector.max(cands[:, cb + 8 : cb + 16], Dt[:, 256:512])
                    if u == 0 and pending is not None:
                        merge(*pending)
                        if t == TILES - 1:
                            # tiles 0..30 are final: clamp now so their Ln
                            # can overlap the last tile's DVE chain
                            nc.vector.tensor_scalar_min(
                                selc[:, 0:496], selbuf[:, 0:496], CLAMP
                            )
                        pending = None
                pending = (t, cands)
            nc.scalar.activation(
                logs[:, 0:496], selc[:, 0:496],
                mybir.ActivationFunctionType.Ln, bias=shiftc[:], scale=-1.0,
            )
            merge(*pending)
            nc.vector.tensor_scalar_min(selc[:, 496:512], selbuf[:, 496:512],
                                        CLAMP)
            nc.scalar.activation(
                logs[:, 496:512], selc[:, 496:512],
                mybir.ActivationFunctionType.Ln, bias=shiftc[:], scale=-1.0,
            )
            logs3 = logs[:].rearrange("p (g j) -> p g j", j=16)
            rsum = persist.tile([128, TILES], f32)
            nc.vector.tensor_reduce(
                rsum[:], logs3, axis=mybir.AxisListType.X,
                op=mybir.AluOpType.add,
            )
            t15 = persist.tile([128, TILES], f32)
            nc.vector.tensor_scalar_mul(t15[:], logs3[:, :, 15], float(KNN - 1))
            acc = persist.tile([128, TILES], f32)
            nc.vector.tensor_sub(acc[:], t15[:], rsum[:])
            nc.sync.dma_start(out_d.ap()[:], acc[:])

    nc.compile()
    return nc


def get_compiled():
    global _compiled
    if _compiled is None:
        _compiled = _build()
    return _compiled


def _split(a):
    hi = a.astype(BF16)
    lo = (a - hi.astype(np.float64)).astype(BF16)
    return hi, lo


def prep_inputs(X):
    """X [B, N, D] f32 -> per-core {xt, qt} maps (y = SHIFT - d2 layout)."""
    in_maps = []
    for c in range(NCORES):
        b, h = c // 2, c % 2
        Xb = np.ascontiguousarray(X[b])                       # [N, D] f32
        sqx = (Xb.astype(np.float64) ** 2).sum(1)             # [N] f64
        Xhi, Xlo = _split(Xb)
        nhi, nlo = _split(-sqx)

        xt = np.zeros([128, N], BF16)
        xt[0:32] = (Xhi.astype(np.float32) * 2.0).astype(BF16).T
        xt[32:64] = (Xlo.astype(np.float32) * 2.0).astype(BF16).T
        xt[64:96] = xt[0:32]
        xt[96] = nhi
        xt[97] = nlo
        xt[98] = BF16(1.0)
        xt[99] = BF16(1.0)

        Qb = Xb[h * ROWS_PER_CORE : (h + 1) * ROWS_PER_CORE]  # [4096, D]
        sqq = sqx[h * ROWS_PER_CORE : (h + 1) * ROWS_PER_CORE]
        Qhi, Qlo = _split(Qb)
        bhi, blo = _split(SHIFT - sqq)
        qt = np.zeros([128, ROWS_PER_CORE], BF16)
        qt[0:32] = Qhi.T
        qt[32:64] = Qhi.T
        qt[64:96] = Qlo.T
        qt[96] = BF16(1.0)
        qt[97] = BF16(1.0)
        qt[98] = bhi
        qt[99] = blo

        in_maps.append({"xt": xt, "qt": qt})
    return in_maps


def finish(acc_list):
    """acc_list: per-core [128, TILES] f32 of 15*L16 - sum_{j=0..15} L_j.

    2*s_i = acc_i + ln(0.25)  (rank-0 is the clamped self distance).
    out_b = 2*(k-2)*N / sum_i 2*s_i ... i.e. (k-2)*N / sum s_i.
    """
    S = np.zeros(B, np.float64)
    for c, a in enumerate(acc_list):
        S[c // 2] += (a.astype(np.float64) + LN_SELF).sum()
    return (2.0 * (KNN - 2) * N / S).astype(np.float32)


def kernel(X, k):
    assert int(k) == KNN
    X = np.asarray(X, dtype=np.float32)
    assert X.shape == (B, N, D)
    nc = get_compiled()
    in_maps = prep_inputs(X)
    # The axon tunnel occasionally throws a transient
    # NRT_EXEC_UNIT_UNRECOVERABLE on execute; a retry reliably recovers.
    last_err = None
    for _ in range(3):
        try:
            res = run_bass_kernel_spmd(nc, in_maps, list(range(NCORES)))
            acc_list = [res.results[c]["acc_out"] for c in range(NCORES)]
            return finish(acc_list)
        except Exception as e:  # noqa: BLE001 - device transients surface broadly
            last_err = e
    raise last_err
